# revision 33
# baseline (speedup 1.0000x reference)
"""Trainium2 (8 NeuronCores) kernel for a dense causal multi-head attention block.

Problem shapes: B=2, S=2048, D=2048, H=16, DH=128 (fp32 in/out).

Distribution strategy (sharding_hint: tensor-parallel over heads):
  Phase 1 (head parallel): core c owns heads {2c, 2c+1}. Per (head, batch)
  section it computes Q^T/K^T/V^T = W^T @ X^T in [DH, S] layout, then causal
  attention fully on-chip:
     scores^T[k, q] = K^T.T @ Q^T          (PE, one matmul per 128x512 tile)
     p = exp(scores / sqrt(DH))            (ACT, straight from PSUM)
     diagonal tiles masked by a 0/1 bf16 mask (DVE)
     z^T[dh, q]  += V_tile.T @ p           (PE, PSUM accumulation over k)
     den_bc[128, q] = ones128.T @ p-sums   (PE rank-reduce; the all-ones lhsT
                                            broadcasts den to all partitions)
     z^T *= reciprocal_approx_fast(den_bc) (DVE custom op; no serial [1,512]
                                            divide, no gpsimd broadcast)
  AllToAll (2 MB bf16) per local head reshards z^T from (head-sharded, all
  rows) to (all heads, 512-row shard).
  Phase 2 (row parallel): out[q, d] = Z^T.T @ W_O + b_O for the core's 512
  rows, split by head parity across the two collectives.

Scheduling: the attention inner loop is exp(ACT)-throughput-bound and the PE
executes strictly in order, so attention stretches 1-3 drip-feed the NEXT
section's projection/transpose matmuls (a generator yielding one PE op at a
time) into their exp bubbles. The last attention stretch is NOT dripped:
its zs stores gate the second AllToAll, and the whole even-head half of the
output projection runs AFTER that collective's trigger — its ~34us of PE
work shields the cross-core NEFF launch skew (measured 25-120us/run) plus
the collective's ~10-12us fixed latency. A few keep-warm pulses bridge the
residual tail; the odd half starts on qt-chunked Z^T gathers.

DMA: every bulk tensor moves as few, large dma_starts (the per-instruction
engine issue cost is ~0.7us regardless of size, so 64 sliced loads of X^T
would cost ~41us of engine time alone). xt/wo are declared pre-tiled so one
AP covers a whole chunk; Z^T gathers use an AP dim-transpose. Queues:
sync=X^T + wq + zs(a2a-input) stores + half the out stores (NEVER the
scalar/ACT ring for zs: those would drain behind queued exp work and delay
the collective), scalar=wk/wv first, then biases + odd Z^T + the other out
stores, gpsimd=hl=1 weights + W_O chunks + collectives (kept near-empty
around each collective so the doorbell isn't queued behind DMA issues).
Outputs store as bf16 (16MB of fp32 stores would exceed the ~358GB/s HBM
budget inside the store window; bf16 fits and costs <0.1% extra error).

The host wrapper shards/casts inputs (bf16), runs the SPMD NEFF on cores
0-7, and concatenates the per-core row slices into the full output.
"""

import numpy as np
import ml_dtypes

import concourse.bass as bass
import concourse.mybir as mybir
import concourse.tile as tile
from concourse import bacc
from concourse.bass import ts
from concourse.bass_utils import run_bass_kernel_spmd
from concourse.masks import make_identity

B, S, D, H, DH = 2, 2048, 2048, 16, 128
NCORES = 8
HL = H // NCORES            # heads per core = 2
QB = (B * S) // NCORES      # output rows per core = 512
P = 128
SC = 512                    # free-dim chunk (PSUM bank = 512 fp32)
NSC = S // SC               # 4
NDT = D // P                # 16 contraction tiles for D
NST = S // P                # 16 sequence tiles of 128
NQT = QB // P               # 4 local q tiles in phase 2
NDC = D // SC               # 4 output-dim chunks
HP = H // 2                 # heads per parity group = 8
SCALE = 1.0 / float(np.sqrt(DH))
LOOKAHEAD = 3               # scores tiles in flight ahead of z matmuls
QC_END_PULL = 10            # drip items pulled at each q-chunk boundary

F32 = mybir.dt.float32
BF16 = mybir.dt.bfloat16


def build_nc():
    nc = bacc.Bacc("TRN2", target_bir_lowering=False, debug=False,
                   num_devices=NCORES)

    # xt = X^T per batch, declared pre-tiled [B, NDT, P, S] (same bytes as
    # [B, D, S]); weights pre-tiled partition-major on the host. wo is
    # parity+chunk-tiled on the host: wo[par][dc][p][j] = W_O rows of head
    # 2j+par, output cols dc*SC:(dc+1)*SC.
    xt = nc.dram_tensor("xt", [B, NDT, P, S], BF16, kind="ExternalInput")
    wq = nc.dram_tensor("wq", [HL, P, NDT, DH], BF16, kind="ExternalInput")
    wk = nc.dram_tensor("wk", [HL, P, NDT, DH], BF16, kind="ExternalInput")
    wv = nc.dram_tensor("wv", [HL, P, NDT, DH], BF16, kind="ExternalInput")
    bq = nc.dram_tensor("bq", [DH, HL], F32, kind="ExternalInput")
    bk = nc.dram_tensor("bk", [DH, HL], F32, kind="ExternalInput")
    bv = nc.dram_tensor("bv", [DH, HL], F32, kind="ExternalInput")
    wo = nc.dram_tensor("wo", [2, NDC, P, HP, SC], BF16, kind="ExternalInput")
    bo = nc.dram_tensor("bo", [1, D], BF16, kind="ExternalInput")
    out = nc.dram_tensor("out", [QB, D], BF16, kind="ExternalOutput")

    Exp = mybir.ActivationFunctionType.Exp

    with tile.TileContext(nc) as tc:
        with (
            tc.tile_pool(name="const", bufs=1) as cpool,
            tc.tile_pool(name="dram", bufs=1, space="DRAM") as dpool,
            tc.tile_pool(name="ps_acc", bufs=3, space="PSUM") as ps_acc,
            tc.tile_pool(name="ps_p2", bufs=2, space="PSUM") as ps_p2,
            tc.tile_pool(name="ps_z", bufs=2, space="PSUM") as ps_z,
            tc.tile_pool(name="ps_den", bufs=1, space="PSUM") as ps_den,
        ):
            bias_sb = {}
            bo_sb = cpool.tile([1, D], BF16)

            # one AllToAll per local head index (a CC op has ~10us fixed
            # latency, so splitting one does NOT pipeline profitably)
            a2a_in = [dpool.tile([NCORES, P, SC], BF16, tag=f"a2a_in{hl}",
                                 name=f"a2a_in{hl}") for hl in range(HL)]
            a2a_out = [dpool.tile([NCORES, P, SC], BF16, tag=f"a2a_out{hl}",
                                  name=f"a2a_out{hl}") for hl in range(HL)]

            with (
                tc.tile_pool(name="qkv", bufs=2) as qkvpool,
                tc.tile_pool(name="small", bufs=4) as spool,
                tc.tile_pool(name="xt1", bufs=1) as xtpool_b1,
            ):
              with tc.tile_pool(name="wpool", bufs=1) as wpool:
                # per-head weight tiles [d_part, d_tile, dh]. wq(h0) rides
                # the sync ring ahead of X^T; wk/wv(h0) ride scalar; the
                # hl=1 set goes on the otherwise-idle gpsimd queue.
                w_sb = []
                for hl in range(HL):
                    per = []
                    for nm, w in (("wq", wq), ("wk", wk), ("wv", wv)):
                        t_sb = wpool.tile([P, NDT, DH], BF16, tag=f"{nm}{hl}")
                        if hl == 0:
                            eng = nc.sync if nm == "wq" else nc.scalar
                        else:
                            eng = nc.gpsimd
                        eng.dma_start(t_sb, w.ap()[hl])
                        per.append(t_sb)
                    w_sb.append(per)
                # biases ride the scalar ring BEHIND wk/wv (they aren't
                # needed until the first PSUM drain, ~30us in)
                for nm, t in (("q", bq), ("k", bk), ("v", bv)):
                    bb = cpool.tile([P, HL], F32, tag=f"b{nm}",
                                    name=f"bias{nm}")
                    nc.scalar.dma_start(bb, t.ap())
                    bias_sb[nm] = bb
                nc.scalar.dma_start(bo_sb, bo.ap())

                # gpsimd-built constants, emitted after the weight DMA issues
                ident = cpool.tile([P, P], BF16)
                make_identity(nc, ident)
                ones_sq = cpool.tile([P, P], BF16)
                nc.gpsimd.memset(ones_sq, 1.0)
                # mask[ki, t] = 1.0 iff ki <= t: causal triangle, diag tiles
                mask = cpool.tile([P, P], BF16)
                nc.gpsimd.memset(mask, 1.0)
                nc.gpsimd.affine_select(
                    out=mask, in_=mask, compare_op=mybir.AluOpType.is_ge,
                    fill=0.0, base=0, pattern=[[1, P]], channel_multiplier=-1,
                )

                XT = {}
                QKV = {}

                def proj_gen(hl, b):
                    """Generator emitting the (hl, b) projections and V
                    transposes one PE instruction per yield, so they can be
                    dripped into the previous section's attention bubbles."""
                    QT = qkvpool.tile([P, S], BF16, tag="qt")
                    KT = qkvpool.tile([P, S], BF16, tag="kt")
                    VT = qkvpool.tile([P, S], BF16, tag="vt", bufs=1)
                    V_kd = qkvpool.tile([P, NST, DH], BF16, tag="vkd")
                    QKV[hl, b] = (QT, KT, V_kd)
                    for pi, (dst, bcol) in enumerate((
                        (QT, bias_sb["q"]), (KT, bias_sb["k"]),
                        (VT, bias_sb["v"]),
                    )):
                        wt = w_sb[hl][pi]
                        for sc in range(NSC):
                            ps = ps_p2.tile([P, SC], F32, tag="p2")
                            for dt_ in range(NDT):
                                nc.tensor.matmul(
                                    ps, lhsT=wt[:, dt_, :],
                                    rhs=XT[b][:, dt_, ts(sc, SC)],
                                    start=(dt_ == 0), stop=(dt_ == NDT - 1),
                                    skip_group_check=True)
                                yield
                            # drain + bias on DVE (keeps ACT free for exp)
                            nc.vector.tensor_scalar_add(
                                dst[:, ts(sc, SC)], ps, bcol[:, hl:hl + 1])
                    for st in range(NST):
                        pst = ps_p2.tile([P, P], BF16, tag="p2")
                        nc.tensor.matmul(pst, lhsT=VT[:, ts(st, P)],
                                         rhs=ident, is_transpose=True,
                                         skip_group_check=True)
                        nc.vector.tensor_copy(V_kd[:, st, :], pst)
                        yield

                def proj_first(hl, b):
                    """First section's projections. Chunk 0 interleaves the
                    Q/K/V chains across 3 PSUM banks so each X^T slice is
                    consumed (3 matmuls) as it lands — the PE tracks the
                    per-slice DMA arrival wave. Later chunks run chain-major
                    (their whole-chunk loads are far ahead by then)."""
                    QT = qkvpool.tile([P, S], BF16, tag="qt")
                    KT = qkvpool.tile([P, S], BF16, tag="kt")
                    VT = qkvpool.tile([P, S], BF16, tag="vt", bufs=1)
                    V_kd = qkvpool.tile([P, NST, DH], BF16, tag="vkd")
                    QKV[hl, b] = (QT, KT, V_kd)
                    dsts = ((QT, bias_sb["q"]), (KT, bias_sb["k"]),
                            (VT, bias_sb["v"]))
                    banks = [ps_acc.tile([P, SC], F32, tag="acc",
                                         name=f"pf0_{i}") for i in range(3)]
                    for dt_ in range(NDT):
                        for pi in range(3):
                            nc.tensor.matmul(
                                banks[pi], lhsT=w_sb[hl][pi][:, dt_, :],
                                rhs=XT[b][:, dt_, ts(0, SC)],
                                start=(dt_ == 0), stop=(dt_ == NDT - 1),
                                skip_group_check=True)
                    for pi, (dst, bcol) in enumerate(dsts):
                        nc.vector.tensor_scalar_add(
                            dst[:, ts(0, SC)], banks[pi], bcol[:, hl:hl + 1])
                    for sc in range(1, NSC):
                        for pi, (dst, bcol) in enumerate(dsts):
                            ps = ps_acc.tile([P, SC], F32, tag="acc",
                                             name=f"pf{sc}_{pi}")
                            for dt_ in range(NDT):
                                nc.tensor.matmul(
                                    ps, lhsT=w_sb[hl][pi][:, dt_, :],
                                    rhs=XT[b][:, dt_, ts(sc, SC)],
                                    start=(dt_ == 0), stop=(dt_ == NDT - 1),
                                    skip_group_check=True)
                            nc.vector.tensor_scalar_add(
                                dst[:, ts(sc, SC)], ps, bcol[:, hl:hl + 1])
                    for st in range(NST):
                        pst = ps_p2.tile([P, P], BF16, tag="p2")
                        nc.tensor.matmul(pst, lhsT=VT[:, ts(st, P)],
                                         rhs=ident, is_transpose=True,
                                         skip_group_check=True)
                        nc.vector.tensor_copy(V_kd[:, st, :], pst)

                def drain(g):
                    for _ in g:
                        pass

                def pull(g, n):
                    if g is None:
                        return
                    for _ in range(n):
                        if next(g, StopIteration) is StopIteration:
                            return

                def attention(hl, b, drip=None, drip_from=0,
                              drip_until=NSC, pull_kt=2,
                              pull_end=QC_END_PULL, den_on_pe=False):
                    """Causal attention for (hl, b); scores pipelined
                    LOOKAHEAD tiles ahead; diagonal tiles at reduced width.
                    One drip item is pulled per kt step, QC_END_PULL per
                    q-chunk boundary, filling PE bubbles left by exp.

                    den_on_pe=True accumulates the softmax denominator on
                    the PE (den_ps += ones.T @ pexp per tile) instead of the
                    DVE dacc chains: for the last (exp-bound) section the
                    PE has slack while the DVE backlog would otherwise
                    delay the final zs store and the collective trigger."""
                    QT, KT, V_kd = QKV[hl, b]
                    for qc in range(NSC):
                        dripping = drip if drip_from <= qc < drip_until \
                            else None
                        z_ps = ps_z.tile([P, SC], F32, tag="z")
                        nkt = 4 * qc + 4
                        pexps = {}
                        # exp-sum accumulators: four short bf16 chains keep
                        # the DVE in 2x mode and off the critical path
                        dacc = [spool.tile([P, SC], BF16, tag=f"dac{c}",
                                           bufs=1, name=f"dac{c}")
                                for c in range(4)]
                        den_ps = None
                        if den_on_pe:
                            den_ps = ps_den.tile([P, SC], F32, tag="den",
                                                 name=f"denps{qc}")

                        def emit_scores(kt, qc=qc, pexps=None, dacc=dacc):
                            j = kt - 4 * qc
                            lo = 128 * j if j >= 0 else 0
                            s_ps = ps_acc.tile([P, SC], F32, tag="acc")
                            nc.tensor.matmul(
                                s_ps[:, :SC - lo], lhsT=KT[:, ts(kt, P)],
                                rhs=QT[:, qc * SC + lo:(qc + 1) * SC],
                                start=True, stop=True)
                            pexp = spool.tile([P, SC], BF16, tag="p", bufs=5)
                            nc.scalar.activation(
                                pexp[:, lo:], s_ps[:, :SC - lo], Exp,
                                bias=0.0, scale=SCALE)
                            if j >= 0:
                                nc.vector.tensor_mul(
                                    pexp[:, lo:lo + P], pexp[:, lo:lo + P],
                                    mask)
                            if den_on_pe:
                                nc.tensor.matmul(
                                    den_ps[:, lo:], lhsT=ones_sq,
                                    rhs=pexp[:, lo:],
                                    start=(kt == 0), stop=(kt == nkt - 1),
                                    skip_group_check=True)
                            else:
                                da = dacc[kt % 4]
                                if kt < 4:
                                    nc.vector.tensor_copy(da[:, lo:],
                                                          pexp[:, lo:])
                                else:
                                    nc.vector.tensor_add(
                                        da[:, lo:], da[:, lo:], pexp[:, lo:])
                            pexps[kt] = (pexp, lo)

                        def emit_den(qc=qc, dacc=dacc):
                            if den_on_pe:
                                rb = spool.tile([P, SC], F32, tag="rb",
                                                bufs=2)
                                nc.vector.reciprocal_approx_fast(out=rb,
                                                                 in_=den_ps)
                                return rb
                            # merge chains pairwise on DVE, then one all-ones
                            # matmul both reduces over k AND broadcasts den
                            # across all 128 partitions.
                            clo = [128 * c if qc == 0 else 0 for c in range(4)]
                            nc.vector.tensor_add(
                                dacc[0][:, clo[1]:], dacc[0][:, clo[1]:],
                                dacc[1][:, clo[1]:])
                            nc.vector.tensor_add(
                                dacc[2][:, clo[3]:], dacc[2][:, clo[3]:],
                                dacc[3][:, clo[3]:])
                            nc.vector.tensor_add(
                                dacc[0][:, clo[2]:], dacc[0][:, clo[2]:],
                                dacc[2][:, clo[2]:])
                            den_bc = ps_den.tile([P, SC], F32, tag="den")
                            nc.tensor.matmul(den_bc, lhsT=ones_sq,
                                             rhs=dacc[0], start=True,
                                             stop=True)
                            rb = spool.tile([P, SC], F32, tag="rb", bufs=2)
                            nc.vector.reciprocal_approx_fast(out=rb,
                                                             in_=den_bc)
                            return rb

                        for k0 in range(min(LOOKAHEAD, nkt)):
                            emit_scores(k0, pexps=pexps)
                        rb = None
                        if nkt <= LOOKAHEAD:
                            rb = emit_den()
                        for kt in range(nkt):
                            pull(dripping, pull_kt)
                            if kt + LOOKAHEAD < nkt:
                                emit_scores(kt + LOOKAHEAD, pexps=pexps)
                                if kt + LOOKAHEAD == nkt - 1:
                                    rb = emit_den()
                            pexp, lo = pexps.pop(kt)
                            nc.tensor.matmul(
                                z_ps[:, lo:], lhsT=V_kd[:, kt, :],
                                rhs=pexp[:, lo:],
                                start=(kt == 0), stop=(kt == nkt - 1),
                                skip_group_check=True)
                        zs = spool.tile([P, SC], BF16, tag="zs", bufs=2)
                        nc.vector.tensor_mul(zs, z_ps, rb)
                        # a2a input stores ride the (idle) sync ring: on the
                        # scalar/ACT ring they'd queue behind the stretch's
                        # remaining exp work and delay the collective trigger
                        nc.sync.dma_start(a2a_in[hl][4 * b + qc], zs)
                        pull(dripping, pull_end)

                # ---------- phase-2 helpers ----------
                p2state = {}

                def p2_open(p2pool):
                    # per-parity Z^T tiles: keeps the odd-half DMA writes
                    # (gated on the 2nd collective) from falsely blocking
                    # even-half reads
                    ZTs = [p2pool.tile([P, HP, SC], BF16, tag=f"zt{par}",
                                       name=f"zt{par}") for par in range(2)]
                    bo_b = p2pool.tile([P, D], BF16, tag="bo_b")
                    nc.gpsimd.partition_broadcast(bo_b, bo_sb)
                    parts = {}
                    for qt in range(NQT):
                        for dc in range(NDC):
                            parts[qt, dc] = p2pool.tile(
                                [P, SC], BF16, tag=f"part{qt}_{dc}",
                                name=f"part{qt}_{dc}")
                    p2state.update(ZTs=ZTs, bo_b=bo_b, parts=parts,
                                   pool=p2pool)
                    p2_load_chunk(0, 0)
                    # even Z^T gather: one dim-transposed AP, one instruction
                    nc.gpsimd.dma_start(ZTs[0][:],
                                        a2a_out[0][:].transpose([1, 0, 2]))

                def p2_load_chunk(par, dc, eng=None):
                    """One 512-col W_O chunk of a parity group = one
                    contiguous dma_start (host pre-tiled); double-buffered
                    per parity so the next chunk prefetches under the
                    current slots."""
                    WOc = p2state["pool"].tile([P, HP, SC], BF16,
                                               tag=f"woc{par}", bufs=2,
                                               name=f"woc{par}_{dc}")
                    p2state["WOc", par, dc] = WOc
                    eng = eng or nc.gpsimd
                    eng.dma_start(WOc, wo.ap()[par][dc])

                def p2_slot(par, dc, qt):
                    """Accumulate 8 parity heads into the (qt, dc) output
                    tile; yields per head."""
                    ZT, WOc = p2state["ZTs"][par], p2state["WOc", par, dc]
                    pa = ps_p2.tile([P, SC], F32, tag="p2")
                    for j in range(HP):
                        nc.tensor.matmul(pa, lhsT=ZT[:, j, ts(qt, P)],
                                         rhs=WOc[:, j, :],
                                         start=(j == 0), stop=(j == HP - 1),
                                         skip_group_check=True)
                        yield
                    if par == 0:
                        nc.vector.tensor_add(
                            p2state["parts"][qt, dc], pa,
                            p2state["bo_b"][:, ts(dc, SC)])
                    else:
                        osb = p2state["pool"].tile([P, SC], BF16,
                                                   tag="osb", bufs=3)
                        nc.vector.tensor_add(osb, pa,
                                             p2state["parts"][qt, dc])
                        # alternate HWDGE rings so the 8MB of bf16 output
                        # stores stream in parallel with slot compute
                        eng = nc.scalar if qt % 2 == 0 else nc.sync
                        eng.dma_start(out.ap()[ts(qt, P), ts(dc, SC)], osb)

                def p2half_gen(par):
                    for dc in range(NDC):
                        if dc + 1 < NDC:
                            p2_load_chunk(par, dc + 1)
                        for qt in range(NQT):
                            yield from p2_slot(par, dc, qt)

                # ---------- phase 1 ----------
                with tc.tile_pool(name="xt0", bufs=1) as xtpool_b0:
                    for b in range(B):
                        pool = xtpool_b0 if b == 0 else xtpool_b1
                        xtt = pool.tile([P, NDT, S], BF16, tag=f"xt{b}",
                                        name=f"xt{b}")
                        if b == 0:
                            # b=0 loads per (dt row, half-S): 2KB descriptor
                            # runs (vs 1KB for column-chunk APs) keep the
                            # rate near HBM peak; the first half paces
                            # chunk 0 per-row and lands chunk 1 before the
                            # PE needs it. All X^T stays on the sync ring: a
                            # second ring's bulk would steal SDMA bandwidth
                            # from the arrival-paced first chunk
                            for hh in range(2):
                                for dt_ in range(NDT):
                                    nc.sync.dma_start(
                                        xtt[:, dt_, ts(hh, S // 2)],
                                        xt.ap()[b][dt_][:, ts(hh, S // 2)])
                        else:
                            for hh in range(2):
                                nc.sync.dma_start(
                                    xtt[:, :, ts(hh, S // 2)],
                                    xt.ap()[b][:, :, ts(hh, S // 2)]
                                    .transpose([1, 0, 2]))
                        XT[b] = xtt

                    proj_first(0, 0)
                    g01 = proj_gen(0, 1)
                    attention(0, 0, drip=g01)
                    drain(g01)
                    g10 = proj_gen(1, 0)
                    attention(0, 1, drip=g10)
                    nc.gpsimd.collective_compute(
                        "AllToAll", mybir.AluOpType.bypass,
                        replica_groups=[list(range(NCORES))],
                        ins=[a2a_in[0][:]], outs=[a2a_out[0][:]],
                    )
                    drain(g10)
                    g11 = proj_gen(1, 1)
                    attention(1, 0, drip=g11)
                drain(g11)
              # wpool + xtpool_b0 closed: their SBUF feeds phase-2 tiles
              with tc.tile_pool(name="p2", bufs=1) as p2pool:
                p2_open(p2pool)
                # NOTE: dripping even-half slots into this stretch was
                # tried and produced an intermittent NaN (slot matmuls
                # land within ~us of the ZT0/W_O arrivals); the even half
                # stays strictly after the stretch
                attention(1, 1)
                nc.gpsimd.collective_compute(
                    "AllToAll", mybir.AluOpType.bypass,
                    replica_groups=[list(range(NCORES))],
                    ins=[a2a_in[1][:]], outs=[a2a_out[1][:]],
                )
                # odd-head Z^T gathers on the scalar ring (idle by now), in
                # qt-column chunks so the first odd slot starts ~1us after
                # the collective lands instead of waiting for the full MB
                for qt in range(NQT):
                    nc.scalar.dma_start(
                        p2state["ZTs"][1][:, :, ts(qt, P)],
                        a2a_out[1][:].transpose([1, 0, 2])[:, :, ts(qt, P)])
                p2_load_chunk(1, 0)
                # the whole even-head half runs AFTER the collective
                # trigger: its ~34us of PE work shields the cross-core
                # launch skew + collective data + odd Z^T gather latency
                drain(p2half_gen(0))
                # a few keep-warm pulses bridge the typical residual gap
                # between the even drain and the collective's arrival
                for _ in range(8):
                    tp = ps_den.tile([P, SC], F32, tag="den")
                    nc.tensor.matmul(tp[:, :64], lhsT=ones_sq,
                                     rhs=ident[:, :64], start=True,
                                     stop=True, skip_group_check=True)
                    tk = spool.tile([P, 64], BF16, tag="tick", bufs=1)
                    nc.vector.tensor_copy(tk, tp[:, :64])
                drain(p2half_gen(1))

    nc.compile()
    return nc


_CACHE = {}


def _get_nc():
    if "nc" not in _CACHE:
        _CACHE["nc"] = build_nc()
    return _CACHE["nc"]


def make_in_maps(resid_pre, W_Q, W_K, W_V, W_O, b_Q, b_K, b_V, b_O):
    bf = ml_dtypes.bfloat16
    x_bf = np.asarray(resid_pre, np.float32).astype(bf)
    # [B, D, S] viewed tiled as [B, NDT, P, S]
    xt = np.ascontiguousarray(x_bf.transpose(0, 2, 1)).reshape(B, NDT, P, S)
    # weights pre-tiled to [H, P, NDT, DH]: w_t[h, p, o, k] = W[h, o*P + p, k]
    def tile_w(W):
        Wb = np.asarray(W, np.float32).astype(bf)
        return np.ascontiguousarray(
            Wb.reshape(H, NDT, P, DH).transpose(0, 2, 1, 3))
    WQ, WK, WV = tile_w(W_Q), tile_w(W_K), tile_w(W_V)
    # wo tiled [2, NDC, P, HP, SC]: wo[par, dc, p, j, c]
    #   = W_O[head 2j+par, dh p, col dc*SC+c]
    WOr = np.asarray(W_O, np.float32).reshape(H, DH, D)
    WOp = np.stack([WOr[0::2], WOr[1::2]])            # [2, HP, P, D]
    WOp = WOp.reshape(2, HP, P, NDC, SC).transpose(0, 3, 2, 1, 4)
    WOp = np.ascontiguousarray(WOp).astype(bf)        # [2, NDC, P, HP, SC]
    bQ = np.ascontiguousarray(np.asarray(b_Q, np.float32).T)  # [DH, H]
    bK = np.ascontiguousarray(np.asarray(b_K, np.float32).T)
    bV = np.ascontiguousarray(np.asarray(b_V, np.float32).T)
    bO = np.ascontiguousarray(
        np.asarray(b_O, np.float32)).reshape(1, D).astype(bf)
    in_maps = []
    for c in range(NCORES):
        hs = slice(c * HL, (c + 1) * HL)
        in_maps.append({
            "xt": xt,
            "wq": np.ascontiguousarray(WQ[hs]),
            "wk": np.ascontiguousarray(WK[hs]),
            "wv": np.ascontiguousarray(WV[hs]),
            "bq": np.ascontiguousarray(bQ[:, hs]),
            "bk": np.ascontiguousarray(bK[:, hs]),
            "bv": np.ascontiguousarray(bV[:, hs]),
            "wo": WOp,
            "bo": bO,
        })
    return in_maps


def assemble(results):
    out = np.empty((B, S, D), np.float32)
    for c in range(NCORES):
        b, r = divmod(c, NCORES // B)  # divmod(c, 4)
        out[b, r * QB:(r + 1) * QB] = np.asarray(results[c]["out"],
                                                 np.float32)
    return out


def kernel(resid_pre, W_Q, W_K, W_V, W_O, b_Q, b_K, b_V, b_O,
           _trace=False, _return_raw=False):
    nc = _get_nc()
    in_maps = make_in_maps(resid_pre, W_Q, W_K, W_V, W_O, b_Q, b_K, b_V, b_O)
    res = run_bass_kernel_spmd(nc, in_maps, core_ids=list(range(NCORES)),
                               trace=_trace)
    out = assemble(res.results)
    if _return_raw:
        return out, res
    return out
